# revision 26
# baseline (speedup 1.0000x reference)
"""Trainium2 Bass kernel for nn_AttentionBlock (B=8, C=128, W=2048).

Reference computation (per batch b):
    q = Wq @ x + bq ; k = Wk @ x + bk ; v = Wv @ x + bv        # [C, W]
    energy[i, j] = sum_c q[c, i] * k[c, j]                     # [W, W]
    attn = softmax(energy, axis=-1)
    out[c, i] = sum_j v[c, j] * attn[i, j]
    return gamma * out + x

Sharding: data-parallel over batch B across the 8 NeuronCores (1 batch
each), tiny weights replicated, no collectives.

Per-core algorithm (E^T layout: softmax axis j on partitions):
    host precomputes A = Wk^T Wq, so energy^T = X^T (A X) + r 1^T + 1 c^T:
      r (= X^T Wk^T bq, per-partition) folds into the G evacuation bias;
      c (per-i) scales softmax numerator and denominator identically and
      is dropped EXACTLY.
    G  = A X + wr 1^T            [c, i]  (replaces both Q and K projections)
    Vt_j = gamma * (X_j^T Wv^T)  [j, c]  (bv recovered via the residual
                                          input: attn rows sum to 1)
    per half h, per key block j:   ET(h,j) = X_j^T G_h   (PE -> PSUM)
                                   PT(h,j) = exp(ET)     (ACT -> SBUF bf16)
    per i-block b: UT(b)[i, c'] = sum_j PT_j[:, b]^T @ [Vt_j | ones]
      -- U and the softmax denominator S (col 128) in ONE accumulation.
    ob = UT[:, :128] * (1/S)  (DVE recip + scale)
    ot = ob + xt[b]           (xt = x^T + gamma*bv, shipped pre-swizzled)
    out^T rows -> DRAM [W, C]; the host transposes back to [C, W].

Engine budget per body (measured): ACT = 32 exps only (~34.8us, the
pacer); PE ~36us (producers 14.7 + accums 18.4 + G 0.9 + Vt 2.7); DVE
evacuations + recip/scale/residual ~19us; Pool: ones memset + h1 DMA.

Scheduling: in-order PE + 2-slot ET rotation means any PE burst longer
than ~2 exp periods stalls ACT.  So consumer accumulations are split
into 8-matmul half-chains and all non-producer PE work is interleaved
between producers by a per-slot budget.  Consumers lag producers by one
half; the next body's input DMAs + G chain are emitted inside phase A
so the exp stream never gaps at body boundaries.  G/V^T/transpose-free
PSUM: et 2x2 banks (pure producer rotation), ut 2x1, misc 2x1.
"""

import numpy as np

B, C, W = 8, 128, 2048
NCORES = 8
JT = W // 128  # 16 key blocks
NH = 2  # query-axis halves
H = W // NH  # 1024
NB = H // 128  # 8 i-blocks per half
UNROLL = 16

# PE-cost estimates (ns) for the budgeted interleave
SLOT_NS = 1086.0  # measured exp-slot cadence
PROD_NS = 460.0
ACCUM_HALF_NS = 575.0
VT_HALF_NS = 340.0
PART1_NS = 900.0

_CACHE = {}


def _build_bass(reps=1, loop=False, ablate=None):
    from contextlib import ExitStack

    import concourse.mybir as mybir
    import concourse.tile as tile
    from concourse import bacc

    f32 = mybir.dt.float32
    bf16 = mybir.dt.bfloat16
    AF = mybir.ActivationFunctionType

    nc = bacc.Bacc(
        "TRN2",
        target_bir_lowering=False,
        debug=False,
        enable_asserts=False,
        num_devices=NCORES,
    )

    # xt = x^T + gamma*bv, pre-swizzled to [p, b, c] with i = b*128 + p
    xt_d = nc.dram_tensor("xt", [C, JT * C], f32, kind="ExternalInput").ap()
    xh_d = nc.dram_tensor("xh", [C, W], bf16, kind="ExternalInput").ap()
    mw_d = nc.dram_tensor("mw", [C, 2 * C], bf16, kind="ExternalInput").ap()
    wb_d = nc.dram_tensor("wb", [C, 2], f32, kind="ExternalInput").ap()
    out_d = nc.dram_tensor("out", [W, C], f32, kind="ExternalOutput").ap()

    with tile.TileContext(nc) as tc, ExitStack() as ctx:
        mwp = ctx.enter_context(tc.tile_pool(name="mwp", bufs=2))
        xhp = ctx.enter_context(tc.tile_pool(name="xhp", bufs=2))
        xtp = ctx.enter_context(tc.tile_pool(name="xtp", bufs=2))
        gsp = ctx.enter_context(tc.tile_pool(name="gsp", bufs=2))
        vtp = ctx.enter_context(tc.tile_pool(name="vtp", bufs=2))
        ptp = ctx.enter_context(tc.tile_pool(name="ptp", bufs=34))
        rcp = ctx.enter_context(tc.tile_pool(name="rcp", bufs=4))
        obp = ctx.enter_context(tc.tile_pool(name="obp", bufs=4))
        outp = ctx.enter_context(tc.tile_pool(name="outp", bufs=4))
        etp = ctx.enter_context(tc.tile_pool(name="etp", bufs=2, space="PSUM"))
        utp = ctx.enter_context(tc.tile_pool(name="utp", bufs=2, space="PSUM"))
        miscp = ctx.enter_context(tc.tile_pool(name="miscp", bufs=2, space="PSUM"))

        def part1_dma(it):
            """Input DMA dispatch only (no PE work) -- placed early so the
            transfers have lead time before the G matmuls need xh."""
            st = {}
            mw = mwp.tile([C, 2 * C], bf16, tag="mw", name=f"mw{it}")
            nc.sync.dma_start(mw, mw_d)
            wb = mwp.tile([C, 2], f32, tag="wb", name=f"wb{it}")
            nc.sync.dma_start(wb, wb_d)
            xh = xhp.tile([C, W], bf16, tag="xh", name=f"xh{it}")
            for chk in range(2):
                sl = slice(chk * 1024, (chk + 1) * 1024)
                nc.sync.dma_start(xh[:, sl], xh_d[:, sl])
            xts = xtp.tile([C, JT, C], f32, tag="xt", name=f"xt{it}")
            for chk in range(2):
                sl = slice(chk * 1024, (chk + 1) * 1024)
                nc.sync.dma_start(
                    xts[:, chk * 8 : (chk + 1) * 8, :], xt_d[:, sl]
                )

            st["xh"], st["xt"] = xh, xts
            st["mwM"] = mw[:, 0:C]        # A^T = Wq^T Wk
            st["mwV"] = mw[:, C : 2 * C]  # Wv^T
            st["wr"] = wb[:, 0:1]         # Wk^T bq
            st["gam"] = wb[:, 1:2]        # gamma
            st["it"] = it
            return st

        def part1_gp(st):
            """G' = A X + wr: 4 x 512-col matmuls through the misc PSUM
            rotation + DVE evacuations."""
            it = st["it"]
            xh = st["xh"]
            gs = gsp.tile([C, W], bf16, tag="gs", name=f"gs{it}")
            st["gs"] = gs
            for m in range(4):
                sl = slice(m * 512, (m + 1) * 512)
                gp = miscp.tile([C, 512], f32, tag="mp", name=f"gp{it}_{m}")
                nc.tensor.matmul(gp, st["mwM"], xh[:, sl], start=True, stop=True)
                nc.vector.tensor_scalar_add(gs[:, sl], gp, st["wr"])
            return st

        def part1(it):
            st = part1_dma(it)
            part1_gp(st)
            return st

        def emit_body(it, st, prev_tail, next_part1):
            xh, xts, gs = st["xh"], st["xt"], st["gs"]
            mwV, gam_col = st["mwV"], st["gam"]

            # vt[:, j, 0:128] = gamma * V^T_j ; vt[:, j, 128] = 1.0
            vt = vtp.tile([C, JT, 129], bf16, tag="vt", name=f"vt{it}")
            vps = {}

            def vt_half(g, second):
                if not second:
                    vp = miscp.tile([C, 512], f32, tag="mp", name=f"vp{it}_{g}")
                    vps[g] = vp
                    for t in range(2):
                        j = 4 * g + t
                        nc.tensor.matmul(
                            vp[:, t * 128 : (t + 1) * 128],
                            xh[:, j * 128 : (j + 1) * 128],
                            mwV,
                            start=True,
                            stop=True,
                        )
                else:
                    vp = vps.pop(g)
                    for t in range(2, 4):
                        j = 4 * g + t
                        nc.tensor.matmul(
                            vp[:, t * 128 : (t + 1) * 128],
                            xh[:, j * 128 : (j + 1) * 128],
                            mwV,
                            start=True,
                            stop=True,
                        )
                    nc.vector.tensor_scalar_mul(
                        vt[:, 4 * g : 4 * (g + 1), 0:128], vp, gam_col
                    )

            def memones():
                nc.gpsimd.memset(vt[:, :, 128:129], 1.0)

            pts = {}

            def prod(h, j):
                et = etp.tile([C, H], f32, tag="et", name=f"et{it}_{h}_{j}")
                for n in range(2):
                    nc.tensor.matmul(
                        et[:, n * 512 : (n + 1) * 512],
                        xh[:, j * 128 : (j + 1) * 128],
                        gs[:, h * H + n * 512 : h * H + (n + 1) * 512],
                        start=True,
                        stop=True,
                    )
                pt = ptp.tile([C, H], bf16, tag="pt", name=f"pt{it}_{h}_{j}")
                nc.scalar.activation(pt, et, AF.Exp)
                pts[(h, j)] = pt

            def make_cons(h):
                """Scheduling units (pe_cost, fn) for half h's consumers:
                accumulations split into 8-matmul half-chains; the finish
                (recip/scale/residual) is PE-free."""
                uts = {}

                def accum_half(b, second):
                    if not second:
                        ut = utp.tile([C, 129], f32, tag="ut",
                                      name=f"ut{it}_{h}_{b}")
                        uts[b] = ut
                        jr = range(0, 8)
                    else:
                        ut = uts[b]
                        jr = range(8, JT)
                    for j in jr:
                        nc.tensor.matmul(
                            ut,
                            pts[(h, j)][:, b * 128 : (b + 1) * 128],
                            vt[:, j, :],
                            start=(j == 0),
                            stop=(j == JT - 1),
                        )

                def finish(b):
                    ut = uts.pop(b)
                    rc = rcp.tile([C, 1], f32, tag="rc", name=f"rc{it}_{h}_{b}")
                    nc.vector.reciprocal_approx_fast(out=rc, in_=ut[:, 128:129])
                    ob = obp.tile([C, 128], f32, tag="ob", name=f"ob{it}_{h}_{b}")
                    nc.vector.tensor_scalar_mul(ob, ut[:, 0:128], rc)
                    gb = h * NB + b
                    ot = outp.tile([C, 128], f32, tag="ot", name=f"ot{it}_{h}_{b}")
                    nc.vector.tensor_add(ot, ob, xts[:, gb, :])
                    pos = slice(gb * 128, (gb + 1) * 128)
                    if h == 0:
                        nc.sync.dma_start(out_d[pos, :], ot)
                    else:
                        nc.gpsimd.dma_start(out_d[pos, :], ot)

                units = []
                for b in range(NB):
                    units.append((ACCUM_HALF_NS,
                                  lambda b=b: accum_half(b, False)))
                    # lag the finish one half-chain so its DVE recip never
                    # head-blocks the queue waiting on the accum's stop
                    if b > 0:
                        units.append((0.0, lambda b=b: finish(b - 1)))
                    units.append((ACCUM_HALF_NS,
                                  lambda b=b: accum_half(b, True)))
                units.append((0.0, lambda: finish(NB - 1)))
                return units

            def sched(prods, units):
                """Greedy budgeted interleave: spend each producer slot's
                spare PE time (SLOT - PROD) on the next units in order."""
                seq = []
                budget = 0.0
                ui = 0
                for p in prods:
                    seq.append(p)
                    budget += SLOT_NS - PROD_NS
                    while ui < len(units) and units[ui][0] <= budget:
                        budget -= units[ui][0]
                        seq.append(units[ui][1])
                        ui += 1
                seq.extend(u[1] for u in units[ui:])
                return seq

            if ablate == "prodexp":
                if next_part1 is not None:
                    next_part1[0]()
                for j in range(JT):
                    prod(0, j)
                if next_part1 is not None:
                    next_part1[1]()
                for j in range(JT):
                    prod(1, j)
                ot = outp.tile([C, 128], f32, tag="ot", name=f"oa{it}")
                nc.vector.tensor_copy(ot, xts[:, 0, :])
                nc.sync.dma_start(out_d[0:128, :], ot)
                return []

            # phase A: h0 producers x (next body's DMA dispatch first for
            # lead time, vt groups 0-1, prev body's h1 consumers)
            unitsA = []
            if next_part1 is not None:
                unitsA.append((0.0, next_part1[0]))
            unitsA += [
                (VT_HALF_NS, lambda: vt_half(0, False)),
                (VT_HALF_NS, lambda: vt_half(0, True)),
                (0.0, memones),
                (VT_HALF_NS, lambda: vt_half(1, False)),
                (VT_HALF_NS, lambda: vt_half(1, True)),
            ]
            unitsA.extend(prev_tail or [])
            for f in sched([lambda h=0, j=j: prod(h, j) for j in range(JT)],
                           unitsA):
                f()
            # phase B: h1 producers x (next body's G chain + vt groups 2-3 +
            # this body's h0 consumers)
            unitsB = []
            if next_part1 is not None:
                unitsB.append((PART1_NS, next_part1[1]))
            unitsB += [
                (VT_HALF_NS, lambda: vt_half(2, False)),
                (VT_HALF_NS, lambda: vt_half(2, True)),
                (VT_HALF_NS, lambda: vt_half(3, False)),
                (VT_HALF_NS, lambda: vt_half(3, True)),
            ]
            unitsB.extend(make_cons(0))
            for f in sched([lambda h=1, j=j: prod(h, j) for j in range(JT)],
                           unitsB):
                f()
            return make_cons(1)

        def emit_chain(n_bodies):
            tail = None
            holder = {"st": part1(0)}
            for u in range(n_bodies):
                if u + 1 < n_bodies:
                    def np_dma(it2=(u + 1) % 2):
                        holder["st_next"] = part1_dma(it2)

                    def np_gp():
                        part1_gp(holder["st_next"])

                    pair = (np_dma, np_gp)
                else:
                    pair = None
                st = holder["st"]
                tail = emit_body(u % 2, st, tail, pair)
                if pair is not None:
                    holder["st"] = holder.pop("st_next")
            for _cost, f in tail:
                f()

        if loop and reps > 1:
            n_iters, rem = divmod(reps, UNROLL)
            with tc.For_i(0, n_iters, 1) as _i:
                emit_chain(UNROLL)
            if rem:
                emit_chain(rem)
        else:
            emit_chain(reps)

    nc.compile()
    return nc


def _get_bass(reps=1, loop=False):
    key = ("nc", reps, loop)
    if key not in _CACHE:
        _CACHE[key] = _build_bass(reps, loop)
    return _CACHE[key]


def _make_in_maps(inputs):
    import ml_dtypes

    f32 = np.float32
    f64 = np.float64
    bf16 = ml_dtypes.bfloat16
    wq = np.asarray(inputs["Wq"], dtype=f64)
    wk = np.asarray(inputs["Wk"], dtype=f64)
    wv = np.asarray(inputs["Wv"], dtype=f64)
    bq = np.asarray(inputs["bq"], dtype=f64).reshape(C)
    bv = np.asarray(inputs["bv"], dtype=f64).reshape(C)
    gm = float(np.asarray(inputs["gamma"], dtype=f64).reshape(()))

    mwM = (wq.T @ wk).astype(bf16)          # A^T, A = Wk^T Wq
    mwV = np.ascontiguousarray(wv.T).astype(bf16)
    mw = np.ascontiguousarray(np.concatenate([mwM, mwV], axis=1))

    wr = (wk.T @ bq).reshape(C, 1)          # Wk^T bq
    gamc = np.full((C, 1), gm, dtype=f64)
    wb = np.ascontiguousarray(np.concatenate([wr, gamc], axis=1).astype(f32))

    xin = np.asarray(inputs["x"], dtype=f32)
    maps = []
    for b in range(B):
        xb = xin[b]
        # xt[p, blk, c] = x[c, blk*128+p] + gamma*bv[c]
        xt = (xb.T.astype(f64) + gm * bv[None, :]).astype(f32)
        xt = np.ascontiguousarray(
            xt.reshape(JT, C, C).transpose(1, 0, 2).reshape(C, JT * C)
        )
        maps.append(
            {
                "xt": xt,
                "xh": np.ascontiguousarray(xb.astype(bf16)),
                "mw": mw,
                "wb": wb,
            }
        )
    return maps


def kernel(x, Wq, bq, Wk, bk, Wv, bv, gamma):
    from concourse import bass_utils

    nc = _get_bass()
    in_maps = _make_in_maps(
        dict(x=x, Wq=Wq, bq=bq, Wk=Wk, bk=bk, Wv=Wv, bv=bv, gamma=gamma)
    )
    res = bass_utils.run_bass_kernel_spmd(nc, in_maps, core_ids=list(range(NCORES)))
    # device returns out^T [W, C]; host restores [C, W]
    return np.stack(
        [np.ascontiguousarray(res.results[b]["out"].T) for b in range(B)], axis=0
    )


# revision 29
# speedup vs baseline: 1.0205x; 1.0205x over previous
"""Trainium2 Bass kernel for nn_AttentionBlock (B=8, C=128, W=2048).

Reference computation (per batch b):
    q = Wq @ x + bq ; k = Wk @ x + bk ; v = Wv @ x + bv        # [C, W]
    energy[i, j] = sum_c q[c, i] * k[c, j]                     # [W, W]
    attn = softmax(energy, axis=-1)
    out[c, i] = sum_j v[c, j] * attn[i, j]
    return gamma * out + x

Sharding: data-parallel over batch B across the 8 NeuronCores (1 batch
each), tiny weights replicated, no collectives.

Per-core algorithm (E^T layout: softmax axis j on partitions):
    host precomputes A = Wk^T Wq, so energy^T = X^T (A X) + r 1^T + 1 c^T:
      r (= X^T Wk^T bq, per-partition) folds into the G evacuation bias;
      c (per-i) scales softmax numerator and denominator identically and
      is dropped EXACTLY.
    G  = A X + wr 1^T            [c, i]  (replaces both Q and K projections)
    Vt_j = gamma * (X_j^T Wv^T)  [j, c]  (bv recovered via the residual
                                          input: attn rows sum to 1)
    per half h, per key block j:   ET(h,j) = X_j^T G_h   (PE -> PSUM)
                                   PT(h,j) = exp(ET)     (ACT -> SBUF bf16)
    per i-block b: UT(b)[i, c'] = sum_j PT_j[:, b]^T @ [Vt_j | ones]
      -- U and the softmax denominator S (col 128) in ONE accumulation.
    ob = UT[:, :128] * (1/S)  (DVE recip + scale)
    ot = ob + xt[b]           (xt = x^T + gamma*bv, shipped pre-swizzled)
    out^T rows -> DRAM [W, C]; the host transposes back to [C, W].

Engine budget per body (measured): ACT = 32 exps only (~34.8us, the
pacer); PE ~36us (producers 14.7 + accums 18.4 + G 0.9 + Vt 2.7); DVE
evacuations + recip/scale/residual ~19us; Pool: ones memset + h1 DMA.

Scheduling: in-order PE + 2-slot ET rotation means any PE burst longer
than ~2 exp periods stalls ACT.  So consumer accumulations are split
into 8-matmul half-chains and all non-producer PE work is interleaved
between producers by a per-slot budget.  Consumers lag producers by one
half; the next body's input DMAs + G chain are emitted inside phase A
so the exp stream never gaps at body boundaries.  G/V^T/transpose-free
PSUM: et 2x2 banks (pure producer rotation), ut 2x1, misc 2x1.
"""

import numpy as np

B, C, W = 8, 128, 2048
NCORES = 8
JT = W // 128  # 16 key blocks
NH = 2  # query-axis halves
H = W // NH  # 1024
NB = H // 128  # 8 i-blocks per half
UNROLL = 16

# PE-cost estimates (ns) for the budgeted interleave
SLOT_NS = 1086.0  # measured exp-slot cadence
PROD_NS = 460.0
ACCUM_HALF_NS = 575.0
VT_HALF_NS = 340.0
PART1_NS = 900.0

_CACHE = {}


def _build_bass(reps=1, loop=False, ablate=None):
    from contextlib import ExitStack

    import concourse.mybir as mybir
    import concourse.tile as tile
    from concourse import bacc

    f32 = mybir.dt.float32
    bf16 = mybir.dt.bfloat16
    AF = mybir.ActivationFunctionType

    nc = bacc.Bacc(
        "TRN2",
        target_bir_lowering=False,
        debug=False,
        enable_asserts=False,
        num_devices=NCORES,
    )

    # xt = x^T + gamma*bv, pre-swizzled to [p, b, c] with i = b*128 + p
    xt_d = nc.dram_tensor("xt", [C, JT * C], f32, kind="ExternalInput").ap()
    xh_d = nc.dram_tensor("xh", [C, W], bf16, kind="ExternalInput").ap()
    mw_d = nc.dram_tensor("mw", [C, 2 * C], bf16, kind="ExternalInput").ap()
    wb_d = nc.dram_tensor("wb", [C, 2], f32, kind="ExternalInput").ap()
    out_d = nc.dram_tensor("out", [W, C], f32, kind="ExternalOutput").ap()

    with tile.TileContext(nc) as tc, ExitStack() as ctx:
        mwp = ctx.enter_context(tc.tile_pool(name="mwp", bufs=2))
        xhp = ctx.enter_context(tc.tile_pool(name="xhp", bufs=2))
        xtp = ctx.enter_context(tc.tile_pool(name="xtp", bufs=2))
        gsp = ctx.enter_context(tc.tile_pool(name="gsp", bufs=2))
        vtp = ctx.enter_context(tc.tile_pool(name="vtp", bufs=2))
        ptp = ctx.enter_context(tc.tile_pool(name="ptp", bufs=38))
        rcp = ctx.enter_context(tc.tile_pool(name="rcp", bufs=4))
        obp = ctx.enter_context(tc.tile_pool(name="obp", bufs=4))
        outp = ctx.enter_context(tc.tile_pool(name="outp", bufs=4))
        etp = ctx.enter_context(tc.tile_pool(name="etp", bufs=2, space="PSUM"))
        utp = ctx.enter_context(tc.tile_pool(name="utp", bufs=2, space="PSUM"))
        miscp = ctx.enter_context(tc.tile_pool(name="miscp", bufs=2, space="PSUM"))

        def part1_dma(it):
            """Input DMA dispatch only (no PE work) -- placed early so the
            transfers have lead time before the G matmuls need xh."""
            st = {}
            mw = mwp.tile([C, 2 * C], bf16, tag="mw", name=f"mw{it}")
            nc.sync.dma_start(mw, mw_d)
            wb = mwp.tile([C, 2], f32, tag="wb", name=f"wb{it}")
            nc.sync.dma_start(wb, wb_d)
            xh = xhp.tile([C, W], bf16, tag="xh", name=f"xh{it}")
            for chk in range(2):
                sl = slice(chk * 1024, (chk + 1) * 1024)
                nc.sync.dma_start(xh[:, sl], xh_d[:, sl])
            xts = xtp.tile([C, JT, C], f32, tag="xt", name=f"xt{it}")
            for chk in range(2):
                sl = slice(chk * 1024, (chk + 1) * 1024)
                nc.sync.dma_start(
                    xts[:, chk * 8 : (chk + 1) * 8, :], xt_d[:, sl]
                )

            st["xh"], st["xt"] = xh, xts
            st["mwM"] = mw[:, 0:C]        # A^T = Wq^T Wk
            st["mwV"] = mw[:, C : 2 * C]  # Wv^T
            st["wr"] = wb[:, 0:1]         # Wk^T bq
            st["gam"] = wb[:, 1:2]        # gamma
            st["it"] = it
            return st

        def part1_gp(st):
            """G' = A X + wr: 4 x 512-col matmuls through the misc PSUM
            rotation + DVE evacuations."""
            it = st["it"]
            xh = st["xh"]
            gs = gsp.tile([C, W], bf16, tag="gs", name=f"gs{it}")
            st["gs"] = gs
            for m in range(4):
                sl = slice(m * 512, (m + 1) * 512)
                gp = miscp.tile([C, 512], f32, tag="mp", name=f"gp{it}_{m}")
                nc.tensor.matmul(gp, st["mwM"], xh[:, sl], start=True, stop=True)
                nc.vector.tensor_scalar_add(gs[:, sl], gp, st["wr"])
            return st

        def part1(it):
            st = part1_dma(it)
            part1_gp(st)
            return st

        def emit_body(it, st, prev_tail, next_part1):
            xh, xts, gs = st["xh"], st["xt"], st["gs"]
            mwV, gam_col = st["mwV"], st["gam"]

            # vt[:, j, 0:128] = gamma * V^T_j ; vt[:, j, 128] = 1.0
            vt = vtp.tile([C, JT, 129], bf16, tag="vt", name=f"vt{it}")
            vps = {}

            def vt_half(g, second):
                if not second:
                    vp = miscp.tile([C, 512], f32, tag="mp", name=f"vp{it}_{g}")
                    vps[g] = vp
                    for t in range(2):
                        j = 4 * g + t
                        nc.tensor.matmul(
                            vp[:, t * 128 : (t + 1) * 128],
                            xh[:, j * 128 : (j + 1) * 128],
                            mwV,
                            start=True,
                            stop=True,
                        )
                else:
                    vp = vps.pop(g)
                    for t in range(2, 4):
                        j = 4 * g + t
                        nc.tensor.matmul(
                            vp[:, t * 128 : (t + 1) * 128],
                            xh[:, j * 128 : (j + 1) * 128],
                            mwV,
                            start=True,
                            stop=True,
                        )
                    nc.vector.tensor_scalar_mul(
                        vt[:, 4 * g : 4 * (g + 1), 0:128], vp, gam_col
                    )

            def memones():
                nc.gpsimd.memset(vt[:, :, 128:129], 1.0)

            pts = {}

            def prod(h, j):
                et = etp.tile([C, H], f32, tag="et", name=f"et{it}_{h}_{j}")
                for n in range(2):
                    nc.tensor.matmul(
                        et[:, n * 512 : (n + 1) * 512],
                        xh[:, j * 128 : (j + 1) * 128],
                        gs[:, h * H + n * 512 : h * H + (n + 1) * 512],
                        start=True,
                        stop=True,
                    )
                pt = ptp.tile([C, H], bf16, tag="pt", name=f"pt{it}_{h}_{j}")
                nc.scalar.activation(pt, et, AF.Exp)
                pts[(h, j)] = pt

            def make_cons(h):
                """Scheduling units (pe_cost, fn) for half h's consumers:
                accumulations split into 8-matmul half-chains; the finish
                (recip/scale/residual) is PE-free."""
                uts = {}

                def accum_half(b, second):
                    if not second:
                        ut = utp.tile([C, 129], f32, tag="ut",
                                      name=f"ut{it}_{h}_{b}")
                        uts[b] = ut
                        jr = range(0, 8)
                    else:
                        ut = uts[b]
                        jr = range(8, JT)
                    for j in jr:
                        nc.tensor.matmul(
                            ut,
                            pts[(h, j)][:, b * 128 : (b + 1) * 128],
                            vt[:, j, :],
                            start=(j == 0),
                            stop=(j == JT - 1),
                        )

                def finish(b):
                    ut = uts.pop(b)
                    rc = rcp.tile([C, 1], f32, tag="rc", name=f"rc{it}_{h}_{b}")
                    nc.vector.reciprocal_approx_fast(out=rc, in_=ut[:, 128:129])
                    ob = obp.tile([C, 128], f32, tag="ob", name=f"ob{it}_{h}_{b}")
                    nc.vector.tensor_scalar_mul(ob, ut[:, 0:128], rc)
                    gb = h * NB + b
                    ot = outp.tile([C, 128], f32, tag="ot", name=f"ot{it}_{h}_{b}")
                    nc.vector.tensor_add(ot, ob, xts[:, gb, :])
                    pos = slice(gb * 128, (gb + 1) * 128)
                    if h == 0:
                        nc.sync.dma_start(out_d[pos, :], ot)
                    else:
                        nc.gpsimd.dma_start(out_d[pos, :], ot)

                units = []
                for b in range(NB):
                    units.append((ACCUM_HALF_NS,
                                  lambda b=b: accum_half(b, False)))
                    # lag the finish one half-chain so its DVE recip never
                    # head-blocks the queue waiting on the accum's stop
                    if b > 0:
                        units.append((0.0, lambda b=b: finish(b - 1)))
                    units.append((ACCUM_HALF_NS,
                                  lambda b=b: accum_half(b, True)))
                units.append((0.0, lambda: finish(NB - 1)))
                return units

            def sched(prods, units):
                """Greedy budgeted interleave: spend each producer slot's
                spare PE time (SLOT - PROD) on the next units in order.
                Units that don't fit are RETURNED (carried into the next
                phase/body) instead of head-blocking the next producers."""
                seq = []
                budget = 0.0
                ui = 0
                for p in prods:
                    seq.append(p)
                    budget += SLOT_NS - PROD_NS
                    while ui < len(units) and units[ui][0] <= budget:
                        budget -= units[ui][0]
                        seq.append(units[ui][1])
                        ui += 1
                return seq, list(units[ui:])

            if ablate == "prodexp":
                if next_part1 is not None:
                    next_part1[0]()
                for j in range(JT):
                    prod(0, j)
                if next_part1 is not None:
                    next_part1[1]()
                for j in range(JT):
                    prod(1, j)
                ot = outp.tile([C, 128], f32, tag="ot", name=f"oa{it}")
                nc.vector.tensor_copy(ot, xts[:, 0, :])
                nc.sync.dma_start(out_d[0:128, :], ot)
                return []

            # phase A: h0 producers x (next body's DMA dispatch first for
            # lead time, vt groups 0-1, prev body's h1 consumers)
            unitsA = []
            if next_part1 is not None:
                unitsA.append((0.0, next_part1[0]))
            unitsA += [
                (VT_HALF_NS, lambda: vt_half(0, False)),
                (VT_HALF_NS, lambda: vt_half(0, True)),
                (0.0, memones),
                (VT_HALF_NS, lambda: vt_half(1, False)),
                (VT_HALF_NS, lambda: vt_half(1, True)),
            ]
            unitsA.extend(prev_tail or [])
            seqA, leftA = sched(
                [lambda h=0, j=j: prod(h, j) for j in range(JT)], unitsA
            )
            for f in seqA:
                f()
            # phase B: h1 producers x (next body's G chain + vt groups 2-3 +
            # phase A leftovers + this body's h0 consumers)
            unitsB = []
            if next_part1 is not None:
                unitsB.append((PART1_NS, next_part1[1]))
            unitsB += [
                (VT_HALF_NS, lambda: vt_half(2, False)),
                (VT_HALF_NS, lambda: vt_half(2, True)),
                (VT_HALF_NS, lambda: vt_half(3, False)),
                (VT_HALF_NS, lambda: vt_half(3, True)),
            ]
            unitsB.extend(leftA)
            unitsB.extend(make_cons(0))
            seqB, leftB = sched(
                [lambda h=1, j=j: prod(h, j) for j in range(JT)], unitsB
            )
            for f in seqB:
                f()
            return leftB + make_cons(1)

        def emit_chain(n_bodies):
            tail = None
            holder = {"st": part1(0)}
            for u in range(n_bodies):
                if u + 1 < n_bodies:
                    def np_dma(it2=(u + 1) % 2):
                        holder["st_next"] = part1_dma(it2)

                    def np_gp():
                        part1_gp(holder["st_next"])

                    pair = (np_dma, np_gp)
                else:
                    pair = None
                st = holder["st"]
                tail = emit_body(u % 2, st, tail, pair)
                if pair is not None:
                    holder["st"] = holder.pop("st_next")
            for _cost, f in tail:
                f()

        if loop and reps > 1:
            n_iters, rem = divmod(reps, UNROLL)
            with tc.For_i(0, n_iters, 1) as _i:
                emit_chain(UNROLL)
            if rem:
                emit_chain(rem)
        else:
            emit_chain(reps)

    nc.compile()
    return nc


def _get_bass(reps=1, loop=False):
    key = ("nc", reps, loop)
    if key not in _CACHE:
        _CACHE[key] = _build_bass(reps, loop)
    return _CACHE[key]


def _make_in_maps(inputs):
    import ml_dtypes

    f32 = np.float32
    f64 = np.float64
    bf16 = ml_dtypes.bfloat16
    wq = np.asarray(inputs["Wq"], dtype=f64)
    wk = np.asarray(inputs["Wk"], dtype=f64)
    wv = np.asarray(inputs["Wv"], dtype=f64)
    bq = np.asarray(inputs["bq"], dtype=f64).reshape(C)
    bv = np.asarray(inputs["bv"], dtype=f64).reshape(C)
    gm = float(np.asarray(inputs["gamma"], dtype=f64).reshape(()))

    mwM = (wq.T @ wk).astype(bf16)          # A^T, A = Wk^T Wq
    mwV = np.ascontiguousarray(wv.T).astype(bf16)
    mw = np.ascontiguousarray(np.concatenate([mwM, mwV], axis=1))

    wr = (wk.T @ bq).reshape(C, 1)          # Wk^T bq
    gamc = np.full((C, 1), gm, dtype=f64)
    wb = np.ascontiguousarray(np.concatenate([wr, gamc], axis=1).astype(f32))

    xin = np.asarray(inputs["x"], dtype=f32)
    maps = []
    for b in range(B):
        xb = xin[b]
        # xt[p, blk, c] = x[c, blk*128+p] + gamma*bv[c]
        xt = (xb.T.astype(f64) + gm * bv[None, :]).astype(f32)
        xt = np.ascontiguousarray(
            xt.reshape(JT, C, C).transpose(1, 0, 2).reshape(C, JT * C)
        )
        maps.append(
            {
                "xt": xt,
                "xh": np.ascontiguousarray(xb.astype(bf16)),
                "mw": mw,
                "wb": wb,
            }
        )
    return maps


def kernel(x, Wq, bq, Wk, bk, Wv, bv, gamma):
    from concourse import bass_utils

    nc = _get_bass()
    in_maps = _make_in_maps(
        dict(x=x, Wq=Wq, bq=bq, Wk=Wk, bk=bk, Wv=Wv, bv=bv, gamma=gamma)
    )
    res = bass_utils.run_bass_kernel_spmd(nc, in_maps, core_ids=list(range(NCORES)))
    # device returns out^T [W, C]; host restores [C, W]
    return np.stack(
        [np.ascontiguousarray(res.results[b]["out"].T) for b in range(B)], axis=0
    )
